# revision 47
# baseline (speedup 1.0000x reference)
"""Trainium2 Bass kernel for the MGA dense-transformer block (v2).

Reference computation (per batch n):
    qkv = depthwise3(conv1x1(x, w_qkv), w_dw)         # (3D, L)
    q,k,v per head (dh=64), l2-normalized q,k (temp folded into q),
    scores^T = k_hat^T q_hat, softmax over keys (= partitions),
    out = attn @ v, y = conv1x1(out, w_proj)

Sharding over 8 cores: core c -> (batch n = c//2, head group g = c%2 of 4
heads).  Each core computes its 768 qkv channels, runs attention for its 4
heads, and produces a partial projection y_partial (512, 2048).  Host sums
the two partials per batch.

v2 design notes (vs the original baseline):
  * Heads are packed in PAIRS on the 128 partitions (head 2p at rows 0-63,
    head 2p+1 at rows 64-127).  Score matmuls of the two heads are emitted
    interleaved with operands at base partitions 0/64, which makes the PE
    row-tile them CONCURRENTLY (K=64 each, ~2x score throughput).
  * The Act engine is reserved for the exp stream (the true bottleneck:
    L*L*heads/128 rows), plus the conv-phase center-tap muls and the norm
    sqrts which run while exp is not active.  Act table thrash is avoided
    by grouping all Sqrt calls before the Exp stream (Copy/Square live in
    every table set).
  * Depthwise conv taps read the conv1x1 PSUM directly on DVE
    (affine_then_add), no separate PSUM->SBUF drain.
  * exp writes bf16 scores; PV runs bf16 (1 cyc/row), v^T is built bf16 via
    PE transposes with a ones-row appended for the softmax denominator.
  * Softmax division: DVE fast-reciprocal of the denominator row, Pool
    (gpsimd) partition-broadcast, DVE multiply into per-head outn tiles.
  * PSUM is phase-scoped via tile pools: conv (2x4 banks), norms
    (nrm 4 + bcast 2 + transpose 2), attention (scores 3x2 + po 2x1).
"""

from contextlib import ExitStack

import numpy as np

import concourse.bacc as bacc
import concourse.mybir as mybir
import concourse.tile as tile
from concourse.bass_utils import run_bass_kernel_spmd

F32 = mybir.dt.float32
F32R = mybir.dt.float32r
BF16 = mybir.dt.bfloat16
I16 = mybir.dt.int16
AF = mybir.ActivationFunctionType
ALU = mybir.AluOpType

# int16 Schraudolph exp: es_bf16_bits = int16(s * SCH_A + SCH_B); the int16
# value IS the bf16 bit pattern of exp(s) (top 16 bits of the f32 trick).
# Calibrated for round-to-nearest on s in [-1, 1]: max rel err 3.3%.
SCH_A = float(12102203.161561485 / 65536.0)
SCH_B = 16250.40
# chunk-pairs (of 8 per head/lt) whose exp runs on DVE instead of Act
DVE_EXP_CP = ()

N, D, L, H = 4, 512, 2048, 8
DH = D // H          # 64 head dim
HPC = H // 2         # 4 heads per core
NPAIR = 2            # head pairs per core
C = 3 * 256          # 768 shard qkv channels
P = 128
NLT = L // 512       # 4 query tiles
NCH = L // 128       # 16 key chunks
N_CORES = 8


def build_program(debug_dumps=False, repeat=1, phases="all"):
    """phases: 'conv' | 'norm' (conv+norm+vt) | 'attn' (no proj) | 'all' —
    truncated builds used only for phase-level HW timing."""
    nc = bacc.Bacc("TRN2", target_bir_lowering=False, debug=False)
    dbg = {}
    if debug_dumps:
        dbg["dw"] = nc.dram_tensor("dbg_dw", (6, P, L), F32R, kind="ExternalOutput")
        dbg["qn0"] = nc.dram_tensor("dbg_qn0", (P, L), F32R, kind="ExternalOutput")
        dbg["kn0"] = nc.dram_tensor("dbg_kn0", (P, L), F32R, kind="ExternalOutput")
        dbg["vt00"] = nc.dram_tensor("dbg_vt00", (P, NCH, 66), BF16, kind="ExternalOutput")
        dbg["es0"] = nc.dram_tensor("dbg_es0", (P, 2, 512), BF16, kind="ExternalOutput")
        dbg["po0"] = nc.dram_tensor("dbg_po0", (65, 512), F32, kind="ExternalOutput")
        dbg["bcz0"] = nc.dram_tensor("dbg_bcz0", (DH, 512), F32, kind="ExternalOutput")
        dbg["outn0"] = nc.dram_tensor("dbg_outn0", (DH, L), F32R, kind="ExternalOutput")

    x_d = nc.dram_tensor("x", (D, L), BF16, kind="ExternalInput")
    wqkvT_d = nc.dram_tensor("wqkvT", (D, C), BF16, kind="ExternalInput")
    wdw_d = nc.dram_tensor("wdw", (C, 3), F32, kind="ExternalInput")
    wpT_d = nc.dram_tensor("wpT", (HPC, DH, D), F32R, kind="ExternalInput")
    temps_d = nc.dram_tensor("temps", (2, NPAIR), F32, kind="ExternalInput")
    bones_d = nc.dram_tensor("bones", (P, 2), F32R, kind="ExternalInput")
    ohbc_d = nc.dram_tensor("ohbc", (2, P), F32R, kind="ExternalInput")
    onesr_d = nc.dram_tensor("onesr", (P, DH), F32R, kind="ExternalInput")
    identT_d = nc.dram_tensor("identT", (P, DH), BF16, kind="ExternalInput")
    y_d = nc.dram_tensor("y", (D, L), F32, kind="ExternalOutput")

    with tile.TileContext(nc) as tc, ExitStack() as ctx:
        wp = ctx.enter_context(tc.tile_pool(name="w", bufs=1))
        xp = ctx.enter_context(tc.tile_pool(name="xp", bufs=4))
        dwp = ctx.enter_context(tc.tile_pool(name="dwp", bufs=6))
        onp = ctx.enter_context(tc.tile_pool(name="onp", bufs=1))
        vtp = ctx.enter_context(tc.tile_pool(name="vtp", bufs=1))
        sbp = ctx.enter_context(tc.tile_pool(name="sbp", bufs=2))
        esp = ctx.enter_context(tc.tile_pool(name="esp", bufs=4))
        ysp = ctx.enter_context(tc.tile_pool(name="ysp", bufs=2))

        # ---- weights / constants (DMA issue spread across HWDGE queues) ----
        wq_sb = []
        for kc in range(4):
            t = wp.tile([P, C], BF16, tag=f"wq{kc}", name=f"wq{kc}")
            (nc.scalar if kc % 2 else nc.gpsimd).dma_start(
                t[:], wqkvT_d[kc * 128:(kc + 1) * 128, :]
            )
            wq_sb.append(t)
        wdw_sb = []
        for cc in range(6):
            t = wp.tile([P, 3], F32, tag=f"wdw{cc}", name=f"wdw{cc}")
            nc.gpsimd.dma_start(t[:], wdw_d[cc * 128:(cc + 1) * 128, :])
            wdw_sb.append(t)
        wpj_sb = []
        for h in range(HPC):
            t = wp.tile([DH, D], F32R, tag=f"wpj{h}", name=f"wpj{h}")
            nc.gpsimd.dma_start(t[:], wpT_d[h, :, :])
            wpj_sb.append(t)
        temps_sb = wp.tile([2, NPAIR], F32, tag="temps")
        nc.gpsimd.dma_start(temps_sb[:], temps_d[:])
        bones_sb = wp.tile([P, 2], F32R, tag="bones")
        nc.gpsimd.dma_start(bones_sb[:], bones_d[:])
        ohbc_sb = wp.tile([2, P], F32R, tag="ohbc")
        nc.gpsimd.dma_start(ohbc_sb[:], ohbc_d[:])
        identT_sb = wp.tile([P, DH], BF16, tag="identT")
        nc.gpsimd.dma_start(identT_sb[:], identT_d[:])
        onesr_sb = wp.tile([P, DH], F32R, tag="onesr")
        nc.gpsimd.dma_start(onesr_sb[:], onesr_d[:])

        for rep in range(repeat):
            # ---- load x (bf16, SP queue, issued first-thing) ---------------
            x_sb = []
            for kc in range(4):
                t = xp.tile([P, L], BF16, tag="x", name=f"x{rep}_{kc}")
                nc.sync.dma_start(t[:], x_d[kc * 128:(kc + 1) * 128, :])
                x_sb.append(t)

            dw_sb = {}
            vt_sb = {}

            def conv_chunk(cc, cvp):
                """conv1x1 + depthwise for channel chunk cc (coarse drains)."""
                pre = cvp.tile([P, L], F32, tag="pre", name=f"pre{rep}_{cc}")
                if cc == 0:
                    # kc-outer: streams with the x DMAs
                    for kc in range(4):
                        for lt in range(NLT):
                            nc.tensor.matmul(
                                pre[:, lt * 512:(lt + 1) * 512],
                                wq_sb[kc][:, cc * 128:(cc + 1) * 128],
                                x_sb[kc][:, lt * 512:(lt + 1) * 512],
                                start=(kc == 0), stop=(kc == 3),
                            )
                else:
                    for lt in range(NLT):
                        for kc in range(4):
                            nc.tensor.matmul(
                                pre[:, lt * 512:(lt + 1) * 512],
                                wq_sb[kc][:, cc * 128:(cc + 1) * 128],
                                x_sb[kc][:, lt * 512:(lt + 1) * 512],
                                start=(kc == 0), stop=(kc == 3),
                            )
                dw = dwp.tile([P, L], F32R, tag="dw", name=f"dw{rep}_{cc}")
                nc.scalar.mul(dw[:], pre[:], wdw_sb[cc][:, 1:2])
                nc.vector.affine_then_add(
                    dw[:, 1:L], pre[:, 0:L - 1], dw[:, 1:L],
                    scale=wdw_sb[cc][:, 0:1], bias=0.0,
                )
                nc.vector.affine_then_add(
                    dw[:, 0:L - 1], pre[:, 1:L], dw[:, 0:L - 1],
                    scale=wdw_sb[cc][:, 2:3], bias=0.0,
                )
                dw_sb[cc] = dw
                if debug_dumps:
                    nc.sync.dma_start(dbg["dw"][cc], dw[:])

            def norm_qk(p, t_idx, nps, bcp):
                src = dw_sb[3 * p + t_idx]
                sq = sbp.tile([P, L], F32R, tag="sq", bufs=1,
                              name=f"sq{rep}_{p}{t_idx}")
                nq = sbp.tile([2, L], F32, tag="nq", bufs=1,
                              name=f"nq{rep}_{p}{t_idx}")
                rq = sbp.tile([2, L], F32R, tag="rq", bufs=1,
                              name=f"rq{rep}_{p}{t_idx}")
                nc.vector.tensor_mul(sq[:], src[:], src[:])
                nrm2 = nps.tile([2, L], F32, tag="nrm",
                                name=f"nrm{rep}_{p}{t_idx}")
                for lt in range(NLT):
                    nc.tensor.matmul(
                        nrm2[:, lt * 512:(lt + 1) * 512],
                        bones_sb[:],
                        sq[:, lt * 512:(lt + 1) * 512],
                        start=True, stop=True,
                    )
                nc.vector.reciprocal_approx_fast(nq[:], nrm2[:])
                # rq = sqrt(temp^2/n^2) = temp/||q|| — written f32r so it can
                # feed the f32r broadcast matmul directly
                nc.scalar.activation(
                    rq[:], nq[:], AF.Sqrt,
                    scale=temps_sb[:, p:p + 1] if t_idx == 0 else 1.0,
                )
                for lt in range(NLT):
                    bc = bcp.tile([P, 512], F32, tag="bc",
                                  name=f"bc{rep}_{p}{t_idx}{lt}")
                    nc.tensor.matmul(
                        bc[:], ohbc_sb[:], rq[:, lt * 512:(lt + 1) * 512],
                        start=True, stop=True,
                    )
                    nc.vector.tensor_mul(
                        src[:, lt * 512:(lt + 1) * 512],
                        src[:, lt * 512:(lt + 1) * 512],
                        bc[:],
                    )

            def build_vt(p, s, tpp):
                vt = vtp.tile([P, NCH, 66], BF16, tag=f"vt{p}{s}",
                              name=f"vt{rep}_{p}{s}")
                nc.vector.memset(vt[:, :, 64:66], 1.0)
                b = 64 * s
                vbf = vbf_sb[p]
                for lg in range(NCH // 8):
                    tps = tpp.tile([P, 8, DH], BF16, tag="tps",
                                   name=f"tps{rep}_{p}{s}{lg}")
                    for j in range(8):
                        lc = 8 * lg + j
                        nc.tensor.matmul(
                            tps[:, j, :],
                            vbf[b:b + 64, lc * 128:(lc + 1) * 128],
                            identT_sb[b:b + 64, :],
                            is_transpose=True,
                            start=(j == 0), stop=(j == 7),
                        )
                    nc.scalar.copy(vt[:, 8 * lg:8 * lg + 8, 0:DH], tps[:])
                vt_sb[(p, s)] = vt

            vbf_sb = {}
            # conv phase (coarse v2-style: full-chunk psum ring of 2)
            with tc.tile_pool(name="cvp", bufs=2, space="PSUM") as cvp:
                for cc in range(6):
                    conv_chunk(cc, cvp)
                for p in range(NPAIR):
                    vbf = sbp.tile([P, L], BF16, tag="vbf", name=f"vbf{rep}_{p}")
                    nc.scalar.copy(vbf[:], dw_sb[3 * p + 2][:])
                    vbf_sb[p] = vbf
            if phases == "conv":
                for kc in range(4):
                    nc.sync.dma_start(
                        y_d[kc * 128:(kc + 1) * 128, :].bitcast(F32R),
                        dw_sb[kc][:],
                    )
                continue
            # norm + v^T phase
            with tc.tile_pool(name="nps", bufs=1, space="PSUM") as nps, \
                 tc.tile_pool(name="bcp", bufs=2, space="PSUM") as bcp, \
                 tc.tile_pool(name="tpp", bufs=2, space="PSUM") as tpp:
                for p in range(NPAIR):
                    norm_qk(p, 0, nps, bcp)
                    norm_qk(p, 1, nps, bcp)
                    build_vt(p, 0, tpp)
                    build_vt(p, 1, tpp)
            if phases == "norm":
                for kc in range(4):
                    nc.sync.dma_start(
                        y_d[kc * 128:(kc + 1) * 128, :].bitcast(F32R),
                        dw_sb[kc][:],
                    )
                continue

            if debug_dumps:
                nc.sync.dma_start(dbg["qn0"][:], dw_sb[0][:])
                nc.sync.dma_start(dbg["kn0"][:], dw_sb[1][:])
                nc.sync.dma_start(dbg["vt00"][:], vt_sb[(0, 0)][:])

            # ---- attention phase ------------------------------------------
            outn = [
                onp.tile([DH, L], F32R, tag=f"outn{h}", name=f"outn{rep}_{h}")
                for h in range(HPC)
            ]
            with tc.tile_pool(name="scp", bufs=2, space="PSUM") as scp, \
                 tc.tile_pool(name="pop", bufs=2, space="PSUM") as pop, \
                 tc.tile_pool(name="bczp", bufs=2, space="PSUM") as bczp:
                # software-pipelined emission: PV(cp) is deferred until after
                # scores(cp+1), and each unit's softmax division is deferred
                # into the next unit — the in-order PE queue never heads-of-
                # line blocks on the Act exp stream.
                pend_pv = None     # (p, po pair, cp, es pair)
                pend_div = None    # (p, lt, po pair)

                def emit_pv(job):
                    pp, po_, cp_, es_ = job
                    for s in range(2):
                        for j in range(2):
                            lc = 2 * cp_ + j
                            nc.tensor.matmul(
                                po_[s][:, :],
                                vt_sb[(pp, s)][:, lc, 0:65],
                                es_[s][:, j, :],
                                start=(lc == 0), stop=(lc == NCH - 1),
                            )

                def emit_div(job):
                    pp, lt_, po_ = job
                    if debug_dumps and pp == 0 and lt_ == 0:
                        po_sb = sbp.tile([65, 512], F32, tag="podbg")
                        nc.vector.tensor_copy(po_sb[:], po_[0][:, :])
                        nc.sync.dma_start(dbg["po0"][:], po_sb[:])
                    for s in range(2):
                        # Z row (partition 64) -> SBUF f32r, PE-broadcast
                        # to partitions 0-63, reciprocal, multiply.
                        rec = sbp.tile([P, 512], F32R, tag="rec",
                                       name=f"rec{rep}_{pp}{lt_}{s}")
                        nc.vector.tensor_copy(rec[64:65, :], po_[s][64:65, :])
                        bcz = bczp.tile([DH, 512], F32, tag="bcz",
                                        name=f"bcz{rep}_{pp}{lt_}{s}")
                        nc.tensor.matmul(
                            bcz[:], onesr_sb[64:65, 0:DH], rec[64:65, :],
                            start=True, stop=True,
                        )
                        bczr = sbp.tile([DH, 512], F32, tag="bczr",
                                        name=f"bczr{rep}_{pp}{lt_}{s}")
                        with nc.allow_low_precision(
                            reason="softmax denom feeds f32r mul"
                        ):
                            nc.vector.reciprocal_approx_fast(bczr[:], bcz[:])
                        if debug_dumps and pp == 0 and lt_ == 0 and s == 0:
                            nc.sync.dma_start(dbg["bcz0"][:], bczr[:])
                        nc.vector.tensor_mul(
                            outn[2 * pp + s][:, lt_ * 512:(lt_ + 1) * 512],
                            po_[s][0:DH, :],
                            bczr[:],
                        )

                for p in range(NPAIR):
                    q = dw_sb[3 * p]
                    k = dw_sb[3 * p + 1]
                    for lt in range(NLT):
                        qs = [q[64 * s:64 * s + 64, lt * 512:(lt + 1) * 512]
                              for s in range(2)]
                        po = [
                            pop.tile([65, 512], F32, tag="po",
                                     name=f"po{rep}_{p}{lt}{s}", bufs=2)
                            for s in range(2)
                        ]
                        for cp in range(NCH // 2):
                            sc = [
                                scp.tile([P, 2, 512], F32, tag="sc",
                                         name=f"sc{rep}_{p}{lt}{cp}{s}")
                                for s in range(2)
                            ]
                            # interleave the two heads' score matmuls so the
                            # PE row-tiles them concurrently (K=64 @ base 0/64)
                            for j in range(2):
                                lc = 2 * cp + j
                                for s in range(2):
                                    b = 64 * s
                                    nc.tensor.matmul(
                                        sc[s][:, j, :],
                                        k[b:b + 64, lc * 128:(lc + 1) * 128],
                                        qs[s],
                                        start=True, stop=True,
                                    )
                            es = []
                            for s in range(2):
                                e = esp.tile([P, 2, 512], BF16, tag="es",
                                             name=f"es{rep}_{p}{lt}{cp}{s}")
                                if cp in DVE_EXP_CP:
                                    nc.vector.tensor_scalar(
                                        e[:].bitcast(I16), sc[s][:],
                                        SCH_A, SCH_B, ALU.mult, ALU.add,
                                    )
                                else:
                                    nc.scalar.activation(e[:], sc[s][:], AF.Exp)
                                es.append(e)
                            if debug_dumps and p == 0 and lt == 0 and cp == 0:
                                nc.sync.dma_start(dbg["es0"][:], es[0][:])
                            if pend_pv is not None:
                                emit_pv(pend_pv)
                            if pend_div is not None:
                                emit_div(pend_div)
                                pend_div = None
                            pend_pv = (p, po, cp, es)
                        pend_div = (p, lt, po)
                if pend_pv is not None:
                    emit_pv(pend_pv)
                if pend_div is not None:
                    emit_div(pend_div)
            if debug_dumps:
                nc.sync.dma_start(dbg["outn0"][:], outn[0][:])

            if phases == "attn":
                for h in range(HPC):
                    nc.sync.dma_start(
                        y_d[h * 64:(h + 1) * 64, :].bitcast(F32R), outn[h][:]
                    )
                continue
            # ---- output projection ----------------------------------------
            with tc.tile_pool(name="ypp", bufs=2, space="PSUM") as ypp:
                for oc in range(4):
                    for lt in range(NLT):
                        yps = ypp.tile([P, 512], F32, tag="y",
                                       name=f"y{rep}_{oc}{lt}")
                        for h in range(HPC):
                            nc.tensor.matmul(
                                yps[:],
                                wpj_sb[h][:, oc * 128:(oc + 1) * 128],
                                outn[h][:, lt * 512:(lt + 1) * 512],
                                start=(h == 0), stop=(h == HPC - 1),
                            )
                        ysb = ysp.tile([P, 512], F32, tag="ysb",
                                       name=f"ysb{rep}_{oc}{lt}")
                        if (oc + lt) % 2:
                            nc.scalar.copy(ysb[:], yps[:])
                        else:
                            nc.vector.tensor_copy(ysb[:], yps[:])
                        nc.sync.dma_start(
                            y_d[oc * 128:(oc + 1) * 128, lt * 512:(lt + 1) * 512],
                            ysb[:],
                        )

    nc.compile()
    return nc


def make_in_maps(x, w_qkv, w_dw, w_proj, temperature):
    x = np.asarray(x, dtype=np.float32)
    w_qkv = np.asarray(w_qkv, dtype=np.float32)
    w_dw = np.asarray(w_dw, dtype=np.float32)
    w_proj = np.asarray(w_proj, dtype=np.float32)
    temperature = np.asarray(temperature, dtype=np.float32)

    bones = np.zeros((P, 2), np.float32)
    bones[0:64, 0] = 1.0
    bones[64:128, 1] = 1.0
    ohbc = np.zeros((2, P), np.float32)
    ohbc[0, 0:64] = 1.0
    ohbc[1, 64:128] = 1.0
    import ml_dtypes
    identT = np.vstack([np.eye(DH, dtype=np.float32)] * 2).astype(ml_dtypes.bfloat16)

    in_maps = []
    for c in range(N_CORES):
        n, g = c // 2, c % 2
        # chunk order per core: (pair p, t in q/k/v): rows of the two heads
        rows = np.concatenate([
            off + 256 * g + 128 * p + np.arange(128)
            for p in range(NPAIR)
            for off in (0, 512, 1024)
        ])
        # temps[s, p] = temperature^2 of head 4g + 2p + s (feeds sqrt scale:
        # rq = sqrt(temp^2 / ||q||^2) = temp/||q||)
        temps = np.zeros((2, NPAIR), np.float32)
        for p_ in range(NPAIR):
            for s in range(2):
                temps[s, p_] = temperature[0, 4 * g + 2 * p_ + s, 0, 0] ** 2
        wpT = np.zeros((HPC, DH, D), np.float32)
        for h in range(HPC):
            wpT[h] = w_proj[:, 256 * g + 64 * h: 256 * g + 64 * h + 64, 0].T
        in_maps.append(
            {
                "x": np.ascontiguousarray(x[n]).astype(ml_dtypes.bfloat16),
                "wqkvT": np.ascontiguousarray(w_qkv[rows, :, 0].T).astype(
                    ml_dtypes.bfloat16
                ),
                "wdw": np.ascontiguousarray(w_dw[rows, 0, :]),
                "wpT": np.ascontiguousarray(wpT),
                "temps": temps,
                "bones": bones,
                "ohbc": ohbc,
                "onesr": np.ones((P, DH), np.float32),
                "identT": identT,
            }
        )
    return in_maps


_PROGRAM = None


def _get_program():
    global _PROGRAM
    if _PROGRAM is None:
        _PROGRAM = build_program()
    return _PROGRAM


def kernel(x, w_qkv, w_dw, w_proj, temperature):
    prog = _get_program()
    in_maps = make_in_maps(x, w_qkv, w_dw, w_proj, temperature)
    res = run_bass_kernel_spmd(prog, in_maps, list(range(N_CORES)))
    y = np.empty((N, D, L), np.float32)
    for n in range(N):
        y[n] = res.results[2 * n]["y"] + res.results[2 * n + 1]["y"]
    return y


if __name__ == "__main__":
    prog = build_program()
    print("program built ok")


# revision 50
# speedup vs baseline: 1.1245x; 1.1245x over previous
"""Trainium2 Bass kernel for the MGA dense-transformer block (v2).

Reference computation (per batch n):
    qkv = depthwise3(conv1x1(x, w_qkv), w_dw)         # (3D, L)
    q,k,v per head (dh=64), l2-normalized q,k (temp folded into q),
    scores^T = k_hat^T q_hat, softmax over keys (= partitions),
    out = attn @ v, y = conv1x1(out, w_proj)

Sharding over 8 cores: core c -> (batch n = c//2, head group g = c%2 of 4
heads).  Each core computes its 768 qkv channels, runs attention for its 4
heads, and produces a partial projection y_partial (512, 2048).  Host sums
the two partials per batch.

v2 design notes (vs the original baseline):
  * Heads are packed in PAIRS on the 128 partitions (head 2p at rows 0-63,
    head 2p+1 at rows 64-127).  Score matmuls of the two heads are emitted
    interleaved with operands at base partitions 0/64, which makes the PE
    row-tile them CONCURRENTLY (K=64 each, ~2x score throughput).
  * The Act engine is reserved for the exp stream (the true bottleneck:
    L*L*heads/128 rows), plus the conv-phase center-tap muls and the norm
    sqrts which run while exp is not active.  Act table thrash is avoided
    by grouping all Sqrt calls before the Exp stream (Copy/Square live in
    every table set).
  * Depthwise conv taps read the conv1x1 PSUM directly on DVE
    (affine_then_add), no separate PSUM->SBUF drain.
  * exp writes bf16 scores; PV runs bf16 (1 cyc/row), v^T is built bf16 via
    PE transposes with a ones-row appended for the softmax denominator.
  * Softmax division: DVE fast-reciprocal of the denominator row, Pool
    (gpsimd) partition-broadcast, DVE multiply into per-head outn tiles.
  * PSUM is phase-scoped via tile pools: conv (2x4 banks), norms
    (nrm 4 + bcast 2 + transpose 2), attention (scores 3x2 + po 2x1).
"""

from contextlib import ExitStack

import numpy as np

import concourse.bacc as bacc
import concourse.mybir as mybir
import concourse.tile as tile
from concourse.bass_utils import run_bass_kernel_spmd

F32 = mybir.dt.float32
F32R = mybir.dt.float32r
BF16 = mybir.dt.bfloat16
I16 = mybir.dt.int16
AF = mybir.ActivationFunctionType
ALU = mybir.AluOpType

# int16 Schraudolph exp: es_bf16_bits = int16(s * SCH_A + SCH_B); the int16
# value IS the bf16 bit pattern of exp(s) (top 16 bits of the f32 trick).
# Calibrated for round-to-nearest on s in [-1, 1]: max rel err 3.3%.
SCH_A = float(12102203.161561485 / 65536.0)
SCH_B = 16250.40
# chunk-pairs (of 8 per head/lt) whose exp runs on DVE instead of Act
DVE_EXP_CP = ()

N, D, L, H = 4, 512, 2048, 8
DH = D // H          # 64 head dim
HPC = H // 2         # 4 heads per core
NPAIR = 2            # head pairs per core
C = 3 * 256          # 768 shard qkv channels
P = 128
NLT = L // 512       # 4 query tiles
NCH = L // 128       # 16 key chunks
N_CORES = 8


def build_program(debug_dumps=False, repeat=1, phases="all"):
    """phases: 'conv' | 'norm' (conv+norm+vt) | 'attn' (no proj) | 'all' —
    truncated builds used only for phase-level HW timing."""
    nc = bacc.Bacc("TRN2", target_bir_lowering=False, debug=False)
    dbg = {}
    if debug_dumps:
        dbg["dw"] = nc.dram_tensor("dbg_dw", (6, P, L), F32R, kind="ExternalOutput")
        dbg["qn0"] = nc.dram_tensor("dbg_qn0", (P, L), F32R, kind="ExternalOutput")
        dbg["kn0"] = nc.dram_tensor("dbg_kn0", (P, L), F32R, kind="ExternalOutput")
        dbg["vt00"] = nc.dram_tensor("dbg_vt00", (P, NCH, 66), BF16, kind="ExternalOutput")
        dbg["es0"] = nc.dram_tensor("dbg_es0", (P, 2, 512), BF16, kind="ExternalOutput")
        dbg["po0"] = nc.dram_tensor("dbg_po0", (65, 512), F32, kind="ExternalOutput")
        dbg["bcz0"] = nc.dram_tensor("dbg_bcz0", (DH, 512), F32, kind="ExternalOutput")
        dbg["outn0"] = nc.dram_tensor("dbg_outn0", (DH, L), F32R, kind="ExternalOutput")

    x_d = nc.dram_tensor("x", (D, L), BF16, kind="ExternalInput")
    wqkvT_d = nc.dram_tensor("wqkvT", (D, C), BF16, kind="ExternalInput")
    wdw_d = nc.dram_tensor("wdw", (C, 3), F32, kind="ExternalInput")
    wpT_d = nc.dram_tensor("wpT", (HPC, DH, D), F32R, kind="ExternalInput")
    temps_d = nc.dram_tensor("temps", (2, NPAIR), F32, kind="ExternalInput")
    bones_d = nc.dram_tensor("bones", (P, 2), F32R, kind="ExternalInput")
    ohbc_d = nc.dram_tensor("ohbc", (2, P), F32R, kind="ExternalInput")
    onesr_d = nc.dram_tensor("onesr", (P, DH), F32R, kind="ExternalInput")
    identT_d = nc.dram_tensor("identT", (P, DH), BF16, kind="ExternalInput")
    y_d = nc.dram_tensor("y", (D, L), F32, kind="ExternalOutput")

    with tile.TileContext(nc) as tc, ExitStack() as ctx:
        wp = ctx.enter_context(tc.tile_pool(name="w", bufs=1))
        xp = ctx.enter_context(tc.tile_pool(name="xp", bufs=4))
        dwp = ctx.enter_context(tc.tile_pool(name="dwp", bufs=6))
        onp = ctx.enter_context(tc.tile_pool(name="onp", bufs=1))
        vtp = ctx.enter_context(tc.tile_pool(name="vtp", bufs=1))
        sbp = ctx.enter_context(tc.tile_pool(name="sbp", bufs=2))
        esp = ctx.enter_context(tc.tile_pool(name="esp", bufs=4))
        ysp = ctx.enter_context(tc.tile_pool(name="ysp", bufs=2))

        # ---- weights / constants (DMA issue spread across HWDGE queues) ----
        wq_sb = []
        for kc in range(4):
            t = wp.tile([P, C], BF16, tag=f"wq{kc}", name=f"wq{kc}")
            (nc.scalar if kc % 2 else nc.gpsimd).dma_start(
                t[:], wqkvT_d[kc * 128:(kc + 1) * 128, :]
            )
            wq_sb.append(t)
        wdw_sb = []
        for cc in range(6):
            t = wp.tile([P, 3], F32, tag=f"wdw{cc}", name=f"wdw{cc}")
            nc.gpsimd.dma_start(t[:], wdw_d[cc * 128:(cc + 1) * 128, :])
            wdw_sb.append(t)
        wpj_sb = []
        for h in range(HPC):
            t = wp.tile([DH, D], F32R, tag=f"wpj{h}", name=f"wpj{h}")
            nc.gpsimd.dma_start(t[:], wpT_d[h, :, :])
            wpj_sb.append(t)
        temps_sb = wp.tile([2, NPAIR], F32, tag="temps")
        nc.gpsimd.dma_start(temps_sb[:], temps_d[:])
        bones_sb = wp.tile([P, 2], F32R, tag="bones")
        nc.gpsimd.dma_start(bones_sb[:], bones_d[:])
        ohbc_sb = wp.tile([2, P], F32R, tag="ohbc")
        nc.gpsimd.dma_start(ohbc_sb[:], ohbc_d[:])
        identT_sb = wp.tile([P, DH], BF16, tag="identT")
        nc.gpsimd.dma_start(identT_sb[:], identT_d[:])
        onesr_sb = wp.tile([P, DH], F32R, tag="onesr")
        nc.gpsimd.dma_start(onesr_sb[:], onesr_d[:])

        for rep in range(repeat):
            # ---- load x (bf16, SP queue, issued first-thing) ---------------
            x_sb = []
            for kc in range(4):
                t = xp.tile([P, L], BF16, tag="x", name=f"x{rep}_{kc}")
                nc.sync.dma_start(t[:], x_d[kc * 128:(kc + 1) * 128, :])
                x_sb.append(t)

            dw_sb = {}
            vt_sb = {}

            def fill_chunk(cc, cvp):
                pre = cvp.tile([P, L], F32, tag="pre", name=f"pre{rep}_{cc}")
                if cc == 0:
                    # kc-outer: streams with the x DMAs
                    for kc in range(4):
                        for lt in range(NLT):
                            nc.tensor.matmul(
                                pre[:, lt * 512:(lt + 1) * 512],
                                wq_sb[kc][:, cc * 128:(cc + 1) * 128],
                                x_sb[kc][:, lt * 512:(lt + 1) * 512],
                                start=(kc == 0), stop=(kc == 3),
                            )
                else:
                    for lt in range(NLT):
                        for kc in range(4):
                            nc.tensor.matmul(
                                pre[:, lt * 512:(lt + 1) * 512],
                                wq_sb[kc][:, cc * 128:(cc + 1) * 128],
                                x_sb[kc][:, lt * 512:(lt + 1) * 512],
                                start=(kc == 0), stop=(kc == 3),
                            )
                return pre

            def drain_chunk(cc, pre):
                dw = dwp.tile([P, L], F32R, tag="dw", name=f"dw{rep}_{cc}")
                nc.scalar.mul(dw[:], pre[:], wdw_sb[cc][:, 1:2])
                nc.vector.affine_then_add(
                    dw[:, 1:L], pre[:, 0:L - 1], dw[:, 1:L],
                    scale=wdw_sb[cc][:, 0:1], bias=0.0,
                )
                nc.vector.affine_then_add(
                    dw[:, 0:L - 1], pre[:, 1:L], dw[:, 0:L - 1],
                    scale=wdw_sb[cc][:, 2:3], bias=0.0,
                )
                dw_sb[cc] = dw
                if debug_dumps:
                    nc.sync.dma_start(dbg["dw"][cc], dw[:])

            def norm_qk(p, t_idx, nps, bcp):
                src = dw_sb[3 * p + t_idx]
                sq = sbp.tile([P, L], F32R, tag="sq", bufs=1,
                              name=f"sq{rep}_{p}{t_idx}")
                nq = sbp.tile([2, L], F32, tag="nq", bufs=1,
                              name=f"nq{rep}_{p}{t_idx}")
                rq = sbp.tile([2, L], F32R, tag="rq", bufs=1,
                              name=f"rq{rep}_{p}{t_idx}")
                nc.vector.tensor_mul(sq[:], src[:], src[:])
                nrm2 = nps.tile([2, L], F32, tag="nrm",
                                name=f"nrm{rep}_{p}{t_idx}")
                for lt in range(NLT):
                    nc.tensor.matmul(
                        nrm2[:, lt * 512:(lt + 1) * 512],
                        bones_sb[:],
                        sq[:, lt * 512:(lt + 1) * 512],
                        start=True, stop=True,
                    )
                nc.vector.reciprocal_approx_fast(nq[:], nrm2[:])
                # rq = sqrt(temp^2/n^2) = temp/||q|| — written f32r so it can
                # feed the f32r broadcast matmul directly
                nc.scalar.activation(
                    rq[:], nq[:], AF.Sqrt,
                    scale=temps_sb[:, p:p + 1] if t_idx == 0 else 1.0,
                )
                for lt in range(NLT):
                    bc = bcp.tile([P, 512], F32, tag="bc",
                                  name=f"bc{rep}_{p}{t_idx}{lt}")
                    nc.tensor.matmul(
                        bc[:], ohbc_sb[:], rq[:, lt * 512:(lt + 1) * 512],
                        start=True, stop=True,
                    )
                    nc.vector.tensor_mul(
                        src[:, lt * 512:(lt + 1) * 512],
                        src[:, lt * 512:(lt + 1) * 512],
                        bc[:],
                    )

            def build_vt(p, s, tpp):
                vt = vtp.tile([P, NCH, 66], BF16, tag=f"vt{p}{s}",
                              name=f"vt{rep}_{p}{s}")
                nc.vector.memset(vt[:, :, 64:66], 1.0)
                b = 64 * s
                vbf = vbf_sb[p]
                for lg in range(NCH // 8):
                    tps = tpp.tile([P, 8, DH], BF16, tag="tps",
                                   name=f"tps{rep}_{p}{s}{lg}")
                    for j in range(8):
                        lc = 8 * lg + j
                        nc.tensor.matmul(
                            tps[:, j, :],
                            vbf[b:b + 64, lc * 128:(lc + 1) * 128],
                            identT_sb[b:b + 64, :],
                            is_transpose=True,
                            start=(j == 0), stop=(j == 7),
                        )
                    nc.scalar.copy(vt[:, 8 * lg:8 * lg + 8, 0:DH], tps[:])
                vt_sb[(p, s)] = vt

            vbf_sb = {}
            # conv phase: full-chunk psum ring of 2, drains deferred one
            # chunk so the PE fill stream never queue-blocks on them
            with tc.tile_pool(name="cvp", bufs=2, space="PSUM") as cvp:
                pend_drain = None
                for cc in range(6):
                    pre = fill_chunk(cc, cvp)
                    if pend_drain is not None:
                        drain_chunk(*pend_drain)
                    pend_drain = (cc, pre)
                drain_chunk(*pend_drain)
                for p in range(NPAIR):
                    vbf = sbp.tile([P, L], BF16, tag="vbf", name=f"vbf{rep}_{p}")
                    nc.scalar.copy(vbf[:], dw_sb[3 * p + 2][:])
                    vbf_sb[p] = vbf
            if phases == "conv":
                for kc in range(4):
                    nc.sync.dma_start(
                        y_d[kc * 128:(kc + 1) * 128, :].bitcast(F32R),
                        dw_sb[kc][:],
                    )
                continue
            # norm + v^T phase
            with tc.tile_pool(name="nps", bufs=1, space="PSUM") as nps, \
                 tc.tile_pool(name="bcp", bufs=2, space="PSUM") as bcp, \
                 tc.tile_pool(name="tpp", bufs=2, space="PSUM") as tpp:
                for p in range(NPAIR):
                    norm_qk(p, 0, nps, bcp)
                    norm_qk(p, 1, nps, bcp)
                    build_vt(p, 0, tpp)
                    build_vt(p, 1, tpp)
            if phases == "norm":
                for kc in range(4):
                    nc.sync.dma_start(
                        y_d[kc * 128:(kc + 1) * 128, :].bitcast(F32R),
                        dw_sb[kc][:],
                    )
                continue

            if debug_dumps:
                nc.sync.dma_start(dbg["qn0"][:], dw_sb[0][:])
                nc.sync.dma_start(dbg["kn0"][:], dw_sb[1][:])
                nc.sync.dma_start(dbg["vt00"][:], vt_sb[(0, 0)][:])

            # ---- attention phase ------------------------------------------
            outn = [
                onp.tile([DH, L], F32R, tag=f"outn{h}", name=f"outn{rep}_{h}")
                for h in range(HPC)
            ]
            with tc.tile_pool(name="scp", bufs=2, space="PSUM") as scp, \
                 tc.tile_pool(name="pop", bufs=2, space="PSUM") as pop, \
                 tc.tile_pool(name="bczp", bufs=2, space="PSUM") as bczp:
                # software-pipelined emission: PV(cp) is deferred until after
                # scores(cp+1), and each unit's softmax division is deferred
                # into the next unit — the in-order PE queue never heads-of-
                # line blocks on the Act exp stream.
                pend_pv = None     # (p, po pair, cp, es pair)
                pend_div = None    # (p, lt, po pair)

                def emit_pv(job):
                    pp, po_, cp_, es_ = job
                    for s in range(2):
                        for j in range(2):
                            lc = 2 * cp_ + j
                            nc.tensor.matmul(
                                po_[s][:, :],
                                vt_sb[(pp, s)][:, lc, 0:65],
                                es_[s][:, j, :],
                                start=(lc == 0), stop=(lc == NCH - 1),
                            )

                def emit_div(job):
                    pp, lt_, po_ = job
                    if debug_dumps and pp == 0 and lt_ == 0:
                        po_sb = sbp.tile([65, 512], F32, tag="podbg")
                        nc.vector.tensor_copy(po_sb[:], po_[0][:, :])
                        nc.sync.dma_start(dbg["po0"][:], po_sb[:])
                    for s in range(2):
                        # Z row (partition 64) -> SBUF f32r, PE-broadcast
                        # to partitions 0-63, reciprocal, multiply.
                        rec = sbp.tile([P, 512], F32R, tag="rec",
                                       name=f"rec{rep}_{pp}{lt_}{s}")
                        nc.vector.tensor_copy(rec[64:65, :], po_[s][64:65, :])
                        bcz = bczp.tile([DH, 512], F32, tag="bcz",
                                        name=f"bcz{rep}_{pp}{lt_}{s}")
                        nc.tensor.matmul(
                            bcz[:], onesr_sb[64:65, 0:DH], rec[64:65, :],
                            start=True, stop=True,
                        )
                        bczr = sbp.tile([DH, 512], F32, tag="bczr",
                                        name=f"bczr{rep}_{pp}{lt_}{s}")
                        with nc.allow_low_precision(
                            reason="softmax denom feeds f32r mul"
                        ):
                            nc.vector.reciprocal_approx_fast(bczr[:], bcz[:])
                        if debug_dumps and pp == 0 and lt_ == 0 and s == 0:
                            nc.sync.dma_start(dbg["bcz0"][:], bczr[:])
                        nc.vector.tensor_mul(
                            outn[2 * pp + s][:, lt_ * 512:(lt_ + 1) * 512],
                            po_[s][0:DH, :],
                            bczr[:],
                        )

                for p in range(NPAIR):
                    q = dw_sb[3 * p]
                    k = dw_sb[3 * p + 1]
                    for lt in range(NLT):
                        qs = [q[64 * s:64 * s + 64, lt * 512:(lt + 1) * 512]
                              for s in range(2)]
                        po = [
                            pop.tile([65, 512], F32, tag="po",
                                     name=f"po{rep}_{p}{lt}{s}", bufs=2)
                            for s in range(2)
                        ]
                        for cp in range(NCH // 2):
                            sc = [
                                scp.tile([P, 2, 512], F32, tag="sc",
                                         name=f"sc{rep}_{p}{lt}{cp}{s}")
                                for s in range(2)
                            ]
                            # interleave the two heads' score matmuls so the
                            # PE row-tiles them concurrently (K=64 @ base 0/64)
                            for j in range(2):
                                lc = 2 * cp + j
                                for s in range(2):
                                    b = 64 * s
                                    nc.tensor.matmul(
                                        sc[s][:, j, :],
                                        k[b:b + 64, lc * 128:(lc + 1) * 128],
                                        qs[s],
                                        start=True, stop=True,
                                    )
                            es = []
                            for s in range(2):
                                e = esp.tile([P, 2, 512], BF16, tag="es",
                                             name=f"es{rep}_{p}{lt}{cp}{s}")
                                if cp in DVE_EXP_CP:
                                    nc.vector.tensor_scalar(
                                        e[:].bitcast(I16), sc[s][:],
                                        SCH_A, SCH_B, ALU.mult, ALU.add,
                                    )
                                else:
                                    nc.scalar.activation(e[:], sc[s][:], AF.Exp)
                                es.append(e)
                            if debug_dumps and p == 0 and lt == 0 and cp == 0:
                                nc.sync.dma_start(dbg["es0"][:], es[0][:])
                            if pend_pv is not None:
                                emit_pv(pend_pv)
                            if pend_div is not None:
                                emit_div(pend_div)
                                pend_div = None
                            pend_pv = (p, po, cp, es)
                        pend_div = (p, lt, po)
                if pend_pv is not None:
                    emit_pv(pend_pv)
                if pend_div is not None:
                    emit_div(pend_div)
            if debug_dumps:
                nc.sync.dma_start(dbg["outn0"][:], outn[0][:])

            if phases == "attn":
                for h in range(HPC):
                    nc.sync.dma_start(
                        y_d[h * 64:(h + 1) * 64, :].bitcast(F32R), outn[h][:]
                    )
                continue
            # ---- output projection (drains deferred one tile) --------------
            with tc.tile_pool(name="ypp", bufs=2, space="PSUM") as ypp:
                def proj_drain(oc, lt, yps):
                    ysb = ysp.tile([P, 512], F32, tag="ysb",
                                   name=f"ysb{rep}_{oc}{lt}")
                    if (oc + lt) % 2:
                        nc.scalar.copy(ysb[:], yps[:])
                    else:
                        nc.vector.tensor_copy(ysb[:], yps[:])
                    nc.sync.dma_start(
                        y_d[oc * 128:(oc + 1) * 128, lt * 512:(lt + 1) * 512],
                        ysb[:],
                    )

                pend_y = None
                for oc in range(4):
                    for lt in range(NLT):
                        yps = ypp.tile([P, 512], F32, tag="y",
                                       name=f"y{rep}_{oc}{lt}")
                        for h in range(HPC):
                            nc.tensor.matmul(
                                yps[:],
                                wpj_sb[h][:, oc * 128:(oc + 1) * 128],
                                outn[h][:, lt * 512:(lt + 1) * 512],
                                start=(h == 0), stop=(h == HPC - 1),
                            )
                        if pend_y is not None:
                            proj_drain(*pend_y)
                        pend_y = (oc, lt, yps)
                proj_drain(*pend_y)

    nc.compile()
    return nc


def make_in_maps(x, w_qkv, w_dw, w_proj, temperature):
    x = np.asarray(x, dtype=np.float32)
    w_qkv = np.asarray(w_qkv, dtype=np.float32)
    w_dw = np.asarray(w_dw, dtype=np.float32)
    w_proj = np.asarray(w_proj, dtype=np.float32)
    temperature = np.asarray(temperature, dtype=np.float32)

    bones = np.zeros((P, 2), np.float32)
    bones[0:64, 0] = 1.0
    bones[64:128, 1] = 1.0
    ohbc = np.zeros((2, P), np.float32)
    ohbc[0, 0:64] = 1.0
    ohbc[1, 64:128] = 1.0
    import ml_dtypes
    identT = np.vstack([np.eye(DH, dtype=np.float32)] * 2).astype(ml_dtypes.bfloat16)

    in_maps = []
    for c in range(N_CORES):
        n, g = c // 2, c % 2
        # chunk order per core: (pair p, t in q/k/v): rows of the two heads
        rows = np.concatenate([
            off + 256 * g + 128 * p + np.arange(128)
            for p in range(NPAIR)
            for off in (0, 512, 1024)
        ])
        # temps[s, p] = temperature^2 of head 4g + 2p + s (feeds sqrt scale:
        # rq = sqrt(temp^2 / ||q||^2) = temp/||q||)
        temps = np.zeros((2, NPAIR), np.float32)
        for p_ in range(NPAIR):
            for s in range(2):
                temps[s, p_] = temperature[0, 4 * g + 2 * p_ + s, 0, 0] ** 2
        wpT = np.zeros((HPC, DH, D), np.float32)
        for h in range(HPC):
            wpT[h] = w_proj[:, 256 * g + 64 * h: 256 * g + 64 * h + 64, 0].T
        in_maps.append(
            {
                "x": np.ascontiguousarray(x[n]).astype(ml_dtypes.bfloat16),
                "wqkvT": np.ascontiguousarray(w_qkv[rows, :, 0].T).astype(
                    ml_dtypes.bfloat16
                ),
                "wdw": np.ascontiguousarray(w_dw[rows, 0, :]),
                "wpT": np.ascontiguousarray(wpT),
                "temps": temps,
                "bones": bones,
                "ohbc": ohbc,
                "onesr": np.ones((P, DH), np.float32),
                "identT": identT,
            }
        )
    return in_maps


_PROGRAM = None


def _get_program():
    global _PROGRAM
    if _PROGRAM is None:
        _PROGRAM = build_program()
    return _PROGRAM


def kernel(x, w_qkv, w_dw, w_proj, temperature):
    prog = _get_program()
    in_maps = make_in_maps(x, w_qkv, w_dw, w_proj, temperature)
    res = run_bass_kernel_spmd(prog, in_maps, list(range(N_CORES)))
    y = np.empty((N, D, L), np.float32)
    for n in range(N):
        y[n] = res.results[2 * n]["y"] + res.results[2 * n + 1]["y"]
    return y


if __name__ == "__main__":
    prog = build_program()
    print("program built ok")
